# revision 24
# baseline (speedup 1.0000x reference)
"""Trainium2 Bass kernel for nn_IntraInterLoss (N=65536, D=768, 8 cores).

Math: with n_i = emb_i/||emb_i||, m1 = [target==1], m0 = 1-m1:
  s1 = sum_i m1_i n_i      s0 = sum_i m0_i n_i
  c1 = sum m1              c0 = N - c1
  out = <s1,s0>/(c1 c0) - 0.4 <s1,s1>/c1^2 - 0.1 <s0,s0>/c0^2

Key identity: s1 = w1 @ emb with per-row scalar w1_i = m1_i/||emb_i||, so the
masked normalized sums are two rows of a [2 x 8192] @ [8192 x 768] matmul the
TensorEngine accumulates in PSUM. The device computes [s1, s_all] (lhsT
columns [w1, rs]); the host recovers s0 = s_all - s1 and the count c1 from
target directly. Per 128-row tile:
  - row sumsq via ACT Square+accum (even tiles) / DVE bn_stats (odd tiles)
    to split the elementwise pass across both engines; the bn mean/var
    postprocess runs on the otherwise-idle GpSimd engine
  - rs = 1/sqrt(sumsq): ACT Sqrt + DVE reciprocal (batched per group for
    tiles 0-55; per-tile for the tail so the last-arriving bytes only pay
    one tile's chain)
  - matmul psum[2,:] += [w1|rs].T @ emb_tile (fp32r: 1 PE cycle/row vs 4
    for plain fp32), PSUM-accumulated over all 64 tiles
Pipeline: 7 full 3MiB DMA groups (24KB per-partition runs) + a 6/2-tile
tail; 5-deep buffering keeps the 16 HW DMA engines streaming at the
~370GB/s per-core HBM share, compute hides entirely under the stream.
Data-parallel over 8 cores (8192 rows each); host sums the 8 partial
(s1, s_all) and does the final three 768-dim dots.
Measured: ~80.3us HW exec typical (from 114.9us baseline); rel err ~1.7e-5.
"""

import numpy as np
from contextlib import ExitStack

import concourse.bass as bass
import concourse.bacc as bacc
import concourse.tile as tile
from concourse import mybir
from concourse.bass_utils import run_bass_kernel_spmd

N, D = 65536, 768
N_CORES = 8
SHARD = N // N_CORES          # 8192 rows per core
P = 128                       # SBUF partitions
T = SHARD // P                # 64 tiles of [128, 768] per core
G = 8                         # tiles per DRAM layout block (24KB per run)

F32 = mybir.dt.float32
F32R = mybir.dt.float32r
AF = mybir.ActivationFunctionType

_nc_cache = None


def _build_nc(EMB_BUFS=5, SCR_BUFS=2):
    nc = bacc.Bacc("TRN2", target_bir_lowering=False, debug=False,
                   num_devices=N_CORES)
    # fp32r: same 32-bit floats, but tagged so the PE streams them at 1
    # cycle/row (vs 4 for plain fp32). The BIR verifier requires every
    # producer feeding an fp32r matmul to emit fp32r, so the whole
    # emb path (DRAM -> DMA -> SBUF tile) is declared fp32r and views are
    # bitcast back to f32 for the ACT/DVE sumsq reads.
    emb = nc.dram_tensor("emb", [SHARD, D], F32R, kind="ExternalInput").ap()
    tgt = nc.dram_tensor("tgt", [P, T], F32, kind="ExternalInput").ap()
    out_s = nc.dram_tensor("out_s", [2, D], F32, kind="ExternalOutput").ap()

    # Pipeline schedule: 7 full 8-tile groups (3 MiB DMAs, 24KB runs), then
    # a 6/2 tail so the compute chain hanging off the LAST-arriving bytes
    # is short (~4us instead of ~11us) while keeping runs >= 6KB.
    SCHED = [(g * 8, 8) for g in range(7)] + [(56, 6), (62, 2)]

    with tile.TileContext(nc) as tc, ExitStack() as ctx:
        embp = ctx.enter_context(tc.tile_pool(name="embp", bufs=EMB_BUFS))
        embp6 = ctx.enter_context(tc.tile_pool(name="embp6", bufs=1))
        embp2 = ctx.enter_context(tc.tile_pool(name="embp2", bufs=1))
        scr = ctx.enter_context(tc.tile_pool(name="scr", bufs=SCR_BUFS))
        sing = ctx.enter_context(tc.tile_pool(name="sing", bufs=1))
        psum = ctx.enter_context(tc.tile_pool(name="psum", bufs=1, space="PSUM"))

        tgt_sb = sing.tile([P, T], F32)
        warm = sing.tile([P, 1], F32)
        warm2 = sing.tile([P, 1], F32)

        sumsq = sing.tile([P, T], F32)
        rs = sing.tile([P, T], F32)
        w_all = sing.tile([P, T, 2], F32R)   # col0 = m1/|x|, col1 = 1/|x|
        mv_all = sing.tile([P, T, 2], F32)   # (mean, var) for DVE-path tiles
        tmp_g = sing.tile([P, G], F32)

        acc_a = psum.tile([2, 512], F32)
        acc_b = psum.tile([2, 256], F32)

        # Contiguous-per-partition layout: row = g*(P*G) + p*G + k, i.e.
        # partition p of group g holds rows [g*P*G + p*G, +G) as one
        # contiguous 24KB DRAM run (optimal HBM streaming). Tile (g,k) is
        # rows {p*G+k}, a permutation of the shard; the masked sums are
        # row-order invariant and tgt uses the same layout.
        emb_g = emb.rearrange("(g p k) d -> g p k d", p=P, k=G)

        for idx, (s0, n) in enumerate(SCHED):
            pool = embp if n == 8 else (embp6 if n == 6 else embp2)
            et = pool.tile([P, n, D], F32R)
            g0, o = divmod(s0, G)
            src = emb_g[g0] if n == G else emb_g[g0][:, o:o + n, :]
            if idx == 0:
                # The Scalar engine clears the framework preamble ~1.3us
                # before Sync does; issue the first (stream-critical) DMA
                # from it so HBM streaming starts earlier.
                nc.scalar.dma_start(out=et, in_=src)
            else:
                nc.sync.dma_start(out=et, in_=src)
            if idx == 0:
                # issued after the first emb group so the big stream starts
                # ~0.7us earlier; tgt isn't needed until the first w compute
                nc.sync.dma_start(out=tgt_sb, in_=tgt)
                # Pre-warm the ACT table set: force the Square+Sqrt table
                # loads into the prologue (the Sqrt one otherwise lands on
                # the first group's critical path, ~1.3us).
                nc.vector.memset(warm, 1.0)
                nc.scalar.activation(out=warm2, in_=warm, func=AF.Square)
                nc.scalar.activation(out=warm2, in_=warm, func=AF.Sqrt)
                # Warm the output-DMA path: a queue's first use costs ~8us
                # extra latency; pay it here instead of after the last matmul.
                nc.sync.dma_start(out=out_s[0:1, 0:1], in_=warm2[0:1, 0:1])
            def sumsq_tile(t, j):
                if j % 2 == 0:
                    # ACT path: sumsq[:, j] = sum(x^2) via Square+accum
                    s = scr.tile([P, D], F32, tag="scr_act")
                    nc.scalar.activation(out=s, in_=et[:, t, :].bitcast(F32),
                                         func=AF.Square,
                                         accum_out=sumsq[:, j:j + 1])
                else:
                    # DVE path: bn_stats/bn_aggr -> mean, var;
                    # sum(x^2) = D*(var + mean^2)
                    st = scr.tile([P, 2, 6], F32, tag="scr_bn")
                    er = et[:, t, :].bitcast(F32).rearrange(
                        "p (s f) -> p s f", s=2)
                    nc.vector.bn_stats(out=st[:, 0, :], in_=er[:, 0, :])
                    nc.vector.bn_stats(out=st[:, 1, :], in_=er[:, 1, :])
                    nc.vector.bn_aggr(out=mv_all[:, j, :], in_=st)

            def mv_post(oc, tc_, nodd):
                # mv postprocess on the otherwise-idle GpSimd engine: keeps
                # the DVE queue short (matters at the tail of the pipeline)
                nc.gpsimd.tensor_mul(mv_all[:, oc, 0], mv_all[:, oc, 0],
                                     mv_all[:, oc, 0])
                nc.gpsimd.tensor_add(tmp_g[:, tc_:tc_ + nodd],
                                     mv_all[:, oc, 0], mv_all[:, oc, 1])
                nc.gpsimd.tensor_scalar_mul(sumsq[:, oc],
                                            tmp_g[:, tc_:tc_ + nodd],
                                            float(D))

            def w_cols(cs):
                nc.scalar.activation(out=rs[:, cs], in_=sumsq[:, cs],
                                     func=AF.Sqrt)
                # col1 = 1/|x|; col0 = m1/|x|
                with nc.allow_low_precision(reason="fp32r out == fp32 minus low bits; tol 2e-2"):
                    nc.vector.reciprocal(out=w_all[:, cs, 1], in_=rs[:, cs])
                nc.vector.tensor_mul(w_all[:, cs, 0], tgt_sb[:, cs],
                                     w_all[:, cs, 1].bitcast(F32))

            def mm_tile(t, j):
                lhsT = w_all[:, j, :]
                first = (j == 0)
                last = (j == T - 1)
                nc.tensor.matmul(acc_a[:, :], lhsT, et[:, t, 0:512],
                                 start=first, stop=last,
                                 skip_group_check=True)
                nc.tensor.matmul(acc_b[:, :], lhsT, et[:, t, 512:768],
                                 start=first, stop=last,
                                 skip_group_check=True)

            if s0 < 56:
                # batched: one w-chain per group (fewest instructions)
                for t in range(n):
                    sumsq_tile(t, s0 + t)
                odd = [s0 + t for t in range(n) if (s0 + t) % 2 == 1]
                mv_post(slice(odd[0], odd[-1] + 1, 2), 0, len(odd))
                w_cols(slice(s0, s0 + n))
                for t in range(n):
                    mm_tile(t, s0 + t)
            else:
                # tail: per-tile chains so each tile's matmuls fire as soon
                # as its own sumsq lands — the last-arriving bytes only pay
                # one tile's chain (~3us) instead of a whole group's (~8us)
                for t in range(n):
                    j = s0 + t
                    sumsq_tile(t, j)
                    if j % 2 == 1:
                        mv_post(slice(j, j + 1), j % G, 1)
                    w_cols(slice(j, j + 1))
                    mm_tile(t, j)

        out_s_sb = sing.tile([2, D], F32)
        # parallel PSUM->SBUF drain: ACT takes the 512 block, DVE the 256;
        # the out DMA issues from Scalar (it just wrote the copy - no
        # cross-engine hop, and Sync is busy with teardown waits)
        nc.scalar.copy(out=out_s_sb[:, 0:512], in_=acc_a[:, :])
        nc.vector.tensor_copy(out=out_s_sb[:, 512:768], in_=acc_b[:, :])
        nc.scalar.dma_start(out=out_s, in_=out_s_sb)

    nc.compile()
    return nc


def _get_nc():
    global _nc_cache
    if _nc_cache is None:
        _nc_cache = _build_nc()
    return _nc_cache


def _make_in_maps(emb, target):
    emb = np.ascontiguousarray(np.asarray(emb), dtype=np.float32)
    tgt = np.asarray(target).astype(np.float32)  # values in {0,1}: cast IS the mask
    in_maps = []
    for c in range(N_CORES):
        sh = slice(c * SHARD, (c + 1) * SHARD)
        tgt_t = np.ascontiguousarray(
            tgt[sh].reshape(T // G, P, G).transpose(1, 0, 2).reshape(P, T))
        in_maps.append({"emb": emb[sh], "tgt": tgt_t})
    return in_maps


def run(emb, target, trace=False):
    """Returns (result_scalar_f32, BassKernelResults)."""
    nc = _get_nc()
    target = np.asarray(target)
    in_maps = _make_in_maps(emb, target)
    br = run_bass_kernel_spmd(nc, in_maps, list(range(N_CORES)), trace=trace)
    s = np.zeros((2, D), dtype=np.float64)
    for r in br.results:
        s += r["out_s"].astype(np.float64)
    s1 = s[0]
    s0 = s[1] - s[0]            # device row1 is s_all = s1 + s0
    c1 = float((target == 1).sum())
    c0 = N - c1
    val = (s1 @ s0) / (c1 * c0) - 0.4 * (s1 @ s1) / (c1 * c1) \
        - 0.1 * (s0 @ s0) / (c0 * c0)
    return np.float32(val), br


def kernel(emb, target):
    return run(emb, target)[0]
